# revision 3
# baseline (speedup 1.0000x reference)
"""MoE (8 experts, top-2) Trainium2 kernel.

Strategy (per spec sharding_hint): expert parallelism. The host computes the
(cheap) router — logits, softmax, top-2, renormalized combine weights — and
dispatches each token to the cores owning its two experts ("all-to-all token
dispatch by top-k expert id" done at the sharding step, since kernel() holds
the full inputs host-side). Core e runs the expert-e FFN over its gathered
tokens, capacity-padded so all 8 cores run one SPMD program:

    YT = W2[e]^T @ gelu(W1[e]^T @ XT + b1[e])        (feature-major layouts)

Matmuls run in float32r (fp32 storage, full PE rate for moving dim >= 256).
The host then scatter-adds  (Y + b2[e]) * combine  back into the output.
"""

import os
import sys

import numpy as np

for _p in ("/opt/trn_rl_repo", "/root/.axon_site/_ro/trn_rl_repo"):
    if os.path.isdir(_p) and _p not in sys.path:
        sys.path.insert(0, _p)

NUM_EXPERTS = 8
TOP_K = 2
B, S, H, I = 4, 4096, 1024, 4096
T = B * S
P = 128
NT = 512           # token tile = moving free dim (fp32 max 512)
C_DEFAULT = 4608   # capacity per expert, multiple of NT (seed-0 max count 4302)
KH = H // P        # 8 contraction chunks for stage 1
KI = I // P        # 32 contraction chunks for stage 2

_built = {}        # C -> (nc, input names)


def _build(C):
    import concourse.bacc as bacc
    import concourse.mybir as mybir
    import concourse.tile as tile
    from concourse._compat import get_trn_type

    f32 = mybir.dt.float32
    f32r = mybir.dt.float32r
    GELU = mybir.ActivationFunctionType.Gelu

    nc = bacc.Bacc(
        get_trn_type() or "TRN2",
        target_bir_lowering=False,
        debug=False,
        enable_asserts=False,
    )
    xt = nc.dram_tensor("xt", [H, C], f32r, kind="ExternalInput").ap()
    w1 = nc.dram_tensor("w1", [H, I], f32r, kind="ExternalInput").ap()
    b1 = nc.dram_tensor("b1", [I], f32, kind="ExternalInput").ap()
    w2 = nc.dram_tensor("w2", [I, H], f32r, kind="ExternalInput").ap()
    ya = nc.dram_tensor("ya", [H, C], f32, kind="ExternalOutput").ap()
    yb = nc.dram_tensor("yb", [H, C], f32, kind="ExternalOutput").ap()

    n_t = C // NT

    with tile.TileContext(nc) as tc:
        with (
            tc.tile_pool(name="dram", bufs=1, space="DRAM") as drampool,
            tc.tile_pool(name="bias", bufs=1) as bpool,
        ):
            ht = drampool.tile([KI, P, C], f32r)
            b1sb = bpool.tile([P, KI], f32)
            nc.sync.dma_start(b1sb[:], b1.rearrange("(ib p) -> p ib", p=P))

            # ---- Phase 1: HT[i, t] = gelu(sum_h W1[h, i] * XT[h, t] + b1[i])
            with (
                tc.tile_pool(name="w1p", bufs=1) as w1p,
                tc.tile_pool(name="xp", bufs=2) as xp,
                tc.tile_pool(name="hsp", bufs=4) as hsp,
                tc.tile_pool(name="ps1", bufs=4, space="PSUM") as ps1,
            ):
                w1sb = w1p.tile([P, KH, I], f32r)
                # split the 16.8MB load into KH chunks for DMA parallelism
                w1r = w1.rearrange("(ko p) i -> p ko i", p=P)
                for k in range(KH):
                    nc.sync.dma_start(w1sb[:, k], w1r[:, k])

                for t in range(n_t):
                    xtile = xp.tile([P, KH, NT], f32r, tag="x")
                    nc.sync.dma_start(
                        xtile[:],
                        xt[:, t * NT:(t + 1) * NT].rearrange(
                            "(ko p) n -> p ko n", p=P
                        ),
                    )
                    for ib in range(KI):
                        ps = ps1.tile([P, NT], f32, tag="ps")
                        for k in range(KH):
                            nc.tensor.matmul(
                                ps[:],
                                lhsT=w1sb[:, k, ib * P:(ib + 1) * P],
                                rhs=xtile[:, k],
                                start=(k == 0),
                                stop=(k == KH - 1),
                            )
                        hs = hsp.tile([P, NT], f32r, tag="hs")
                        nc.scalar.activation(
                            hs[:], ps[:], GELU, bias=b1sb[:, ib:ib + 1]
                        )
                        nc.sync.dma_start(ht[ib, :, t * NT:(t + 1) * NT], hs[:])

            # ---- Phase 2: YT[o, t] = sum_i W2[i, o] * HT[i, t], split over i
            with (
                tc.tile_pool(name="w2p", bufs=1) as w2p,
                tc.tile_pool(name="h2p", bufs=2) as h2p,
                tc.tile_pool(name="yp", bufs=4) as yp,
                tc.tile_pool(name="ps2", bufs=4, space="PSUM") as ps2,
            ):
                KHALF = KI // 2  # 16 contraction chunks per half
                for half, yout in ((0, ya), (1, yb)):
                    w2sb = w2p.tile([P, KHALF, H], f32r, tag="w2")
                    w2r = w2[half * KHALF * P:(half + 1) * KHALF * P].rearrange(
                        "(ko p) o -> p ko o", p=P
                    )
                    for k in range(KHALF):
                        nc.sync.dma_start(w2sb[:, k], w2r[:, k])

                    for t in range(n_t):
                        htile = h2p.tile([P, KHALF, NT], f32r, tag="h2")
                        nc.sync.dma_start(
                            htile[:],
                            ht[half * KHALF:(half + 1) * KHALF,
                               :, t * NT:(t + 1) * NT].rearrange(
                                "ko p n -> p ko n"
                            ),
                        )
                        for ob in range(H // P):
                            ps = ps2.tile([P, NT], f32, tag="psy")
                            for k in range(KHALF):
                                nc.tensor.matmul(
                                    ps[:],
                                    lhsT=w2sb[:, k, ob * P:(ob + 1) * P],
                                    rhs=htile[:, k],
                                    start=(k == 0),
                                    stop=(k == KHALF - 1),
                                )
                            ys = yp.tile([P, NT], f32, tag="y")
                            nc.vector.tensor_copy(ys[:], ps[:])
                            nc.sync.dma_start(
                                yout[ob * P:(ob + 1) * P, t * NT:(t + 1) * NT],
                                ys[:],
                            )
    nc.finalize()
    return nc


def _routing(hidden, router_w, router_b):
    """Top-2 routing, bit-matching the jax reference on CPU."""
    import jax
    import jax.numpy as jnp

    cpu = jax.local_devices(backend="cpu")[0]
    with jax.default_device(cpu):
        logits = jnp.einsum("bsh,he->bse", jnp.asarray(hidden),
                            jnp.asarray(router_w)) + jnp.asarray(router_b)
        probs = jax.nn.softmax(logits, axis=-1)
        tkp, tki = jax.lax.top_k(probs, TOP_K)
        tkp = tkp / jnp.sum(tkp, axis=-1, keepdims=True)
        tkp_np = np.asarray(tkp).reshape(T, TOP_K)
        tki_np = np.asarray(tki).reshape(T, TOP_K)
    return tkp_np, tki_np


def kernel(hidden_states, w1, b1, w2, b2, router_w, router_b):
    from concourse import bass_utils

    hidden_states = np.ascontiguousarray(hidden_states, dtype=np.float32)
    w1 = np.ascontiguousarray(w1, dtype=np.float32)
    b1 = np.ascontiguousarray(b1, dtype=np.float32)
    w2 = np.ascontiguousarray(w2, dtype=np.float32)
    b2 = np.ascontiguousarray(b2, dtype=np.float32)

    tkp, tki = _routing(hidden_states, router_w, router_b)
    x = hidden_states.reshape(T, H)

    idx_e, prob_e = [], []
    for e in range(NUM_EXPERTS):
        hit = tki == e                       # [T, 2] bool
        idx = np.nonzero(hit.any(axis=1))[0]
        pe = np.where(hit[idx, 0], tkp[idx, 0], tkp[idx, 1]).astype(np.float32)
        idx_e.append(idx)
        prob_e.append(pe)

    maxn = max(len(ix) for ix in idx_e)
    C = C_DEFAULT if maxn <= C_DEFAULT else ((maxn + NT - 1) // NT) * NT
    if C not in _built:
        _built[C] = _build(C)
    nc = _built[C]

    in_maps = []
    for e in range(NUM_EXPERTS):
        ix = idx_e[e]
        xt = np.zeros((H, C), dtype=np.float32)
        xt[:, :len(ix)] = x[ix].T
        in_maps.append({
            "xt": xt,
            "w1": w1[e],
            "b1": b1[e],
            "w2": w2[e],
        })

    res = bass_utils.run_bass_kernel_spmd(
        nc, in_maps, core_ids=list(range(NUM_EXPERTS))
    ).results

    out = np.zeros((T, H), dtype=np.float32)
    for e in range(NUM_EXPERTS):
        ix = idx_e[e]
        y = (res[e]["ya"][:, :len(ix)] + res[e]["yb"][:, :len(ix)]).T
        out[ix] += (y + b2[e]) * prob_e[e][:, None]
    return out.reshape(B, S, H)


# revision 8
# speedup vs baseline: 2.5935x; 2.5935x over previous
"""MoE (8 experts, top-2) Trainium2 kernel.

Strategy (per spec sharding_hint): expert parallelism. The host computes the
(cheap) router — logits, softmax, top-2, renormalized combine weights — and
dispatches each token to the cores owning its two experts ("all-to-all token
dispatch by top-k expert id" done at the sharding step, since kernel() holds
the full inputs host-side). Core e runs the expert-e FFN over its gathered
tokens, capacity-padded so all 8 cores run one SPMD program:

    YT = W2[e]^T @ gelu(W1[e]^T @ XT + b1[e])        (feature-major layouts)

Matmuls run in float32r (fp32 storage, full PE rate for moving dim >= 256).
The host then scatter-adds  (Y + b2[e]) * combine  back into the output.
"""

import os
import sys

import numpy as np

for _p in ("/opt/trn_rl_repo", "/root/.axon_site/_ro/trn_rl_repo"):
    if os.path.isdir(_p) and _p not in sys.path:
        sys.path.insert(0, _p)

NUM_EXPERTS = 8
TOP_K = 2
B, S, H, I = 4, 4096, 1024, 4096
T = B * S
P = 128
NT = 512           # token tile = moving free dim (fp32 max 512)
C_DEFAULT = 4608   # capacity per expert, multiple of NT (seed-0 max count 4302)
KH = H // P        # 8 contraction chunks for stage 1
KI = I // P        # 32 contraction chunks for stage 2

_built = {}        # C -> (nc, input names)


def _build(C):
    import concourse.bacc as bacc
    import concourse.mybir as mybir
    import concourse.tile as tile
    from concourse._compat import get_trn_type

    f32 = mybir.dt.float32
    f32r = mybir.dt.float32r
    GELU = mybir.ActivationFunctionType.Gelu

    nc = bacc.Bacc(
        get_trn_type() or "TRN2",
        target_bir_lowering=False,
        debug=False,
        enable_asserts=False,
    )
    xt = nc.dram_tensor("xt", [H, C], f32r, kind="ExternalInput").ap()
    w1 = nc.dram_tensor("w1", [H, I], f32r, kind="ExternalInput").ap()
    b1 = nc.dram_tensor("b1", [I], f32, kind="ExternalInput").ap()
    w2 = nc.dram_tensor("w2", [I, H], f32r, kind="ExternalInput").ap()
    ya = nc.dram_tensor("ya", [H, C], f32, kind="ExternalOutput").ap()
    yb = nc.dram_tensor("yb", [H, C], f32, kind="ExternalOutput").ap()

    n_t = C // NT
    IH = I // 2         # 2048: i-range per half-phase
    KIH = KI // 2       # 16 stage-2 contraction chunks per half
    XS = 2              # x sub-tiles per token tile (k-chunks split 2x4)
    HS2 = 4             # h2 sub-tiles per token tile (k-chunks split 4x4)

    with tile.TileContext(nc) as tc:
        with (
            tc.tile_pool(name="dram", bufs=1, space="DRAM") as drampool,
            tc.tile_pool(name="bias", bufs=1) as bpool,
            tc.tile_pool(name="wp", bufs=2) as wp,
            tc.tile_pool(name="xp", bufs=3) as xp,
            tc.tile_pool(name="hsp", bufs=3) as hsp,
            tc.tile_pool(name="h2p", bufs=3) as h2p,
            tc.tile_pool(name="yp", bufs=2) as yp,
            tc.tile_pool(name="psp", bufs=8, space="PSUM") as psp,
        ):
            hta = drampool.tile([KIH, P, C], f32r, tag="hta")
            htb = drampool.tile([KIH, P, C], f32r, tag="htb")
            b1sb = bpool.tile([P, KI], f32)
            nc.sync.dma_start(b1sb[:], b1.rearrange("(ib p) -> p ib", p=P))

            w1r = w1.rearrange("(ko p) i -> p ko i", p=P)
            w2r = w2.rearrange("(ko p) o -> p ko o", p=P)

            for half in range(2):
                ht = hta if half == 0 else htb
                yout = ya if half == 0 else yb

                # -- Phase 1x: HT[i,t] = gelu(W1[:,i]^T XT + b1[i]), i in half
                w1sb = wp.tile([P, KH, IH], f32r, tag="w")

                def _load_w1(lo, hi, half=half, w1sb=w1sb):
                    nc.sync.dma_start(
                        w1sb[:, :, lo:hi],
                        w1r[:, :, half * IH + lo: half * IH + hi],
                    )

                def _load_x(t, s):
                    xst = xp.tile([P, KH // XS, NT], f32r, tag="x",
                                  name=f"x_{half}_{t}_{s}")
                    nc.sync.dma_start(
                        xst[:],
                        xt[(s * KH // XS) * P:((s + 1) * KH // XS) * P,
                           t * NT:(t + 1) * NT].rearrange(
                            "(ko p) n -> p ko n", p=P
                        ),
                    )
                    return xst

                # first w1 column chunk, then t=0 x tiles, then the rest —
                # so PE starts after ~2MB of DMA, not ~20MB
                _load_w1(0, 128)
                xs0 = [_load_x(0, s) for s in range(XS)]
                for lo, hi in ((128, 256), (256, 384), (384, 512),
                               (512, 1024), (1024, 1536), (1536, 2048)):
                    _load_w1(lo, hi)
                for t in range(n_t):
                    xs = xs0 if t == 0 else [_load_x(t, s) for s in range(XS)]
                    for ibl in range(KIH):
                        ib = half * KIH + ibl
                        ps = psp.tile([P, NT], f32, tag="ps")
                        for k in range(KH):
                            nc.tensor.matmul(
                                ps[:],
                                lhsT=w1sb[:, k, ibl * P:(ibl + 1) * P],
                                rhs=xs[k // (KH // XS)][:, k % (KH // XS)],
                                start=(k == 0),
                                stop=(k == KH - 1),
                            )
                        hs = hsp.tile([P, NT], f32r, tag="hs")
                        nc.scalar.activation(
                            hs[:], ps[:], GELU, bias=b1sb[:, ib:ib + 1]
                        )
                        nc.sync.dma_start(ht[ibl, :, t * NT:(t + 1) * NT], hs[:])

                # -- Phase 2x: Y_half[o,t] = sum_{i in half} W2[i,o] HT[i,t]
                w2sb = wp.tile([P, KIH, H], f32r, tag="w")
                for c in range(4):
                    cw = KIH // 4
                    nc.sync.dma_start(
                        w2sb[:, c * cw:(c + 1) * cw],
                        w2r[:, half * KIH + c * cw: half * KIH + (c + 1) * cw],
                    )
                HC = KIH // HS2   # 4 chunks per sub-tile
                for t in range(n_t):
                    pss = []
                    for _ob in range(H // P):
                        pst = psp.tile([P, NT], f32, tag="ps", name=f"ps_{half}_{t}_{_ob}")
                        pss.append(pst)
                    for s in range(HS2):
                        h2t = h2p.tile([P, HC, NT], f32r, tag="h2")
                        nc.sync.dma_start(
                            h2t[:],
                            ht[s * HC:(s + 1) * HC, :,
                               t * NT:(t + 1) * NT].rearrange("ko p n -> p ko n"),
                        )
                        for ob in range(H // P):
                            for kk in range(HC):
                                nc.tensor.matmul(
                                    pss[ob][:],
                                    lhsT=w2sb[:, s * HC + kk,
                                              ob * P:(ob + 1) * P],
                                    rhs=h2t[:, kk],
                                    start=(s == 0 and kk == 0),
                                    stop=(s == HS2 - 1 and kk == HC - 1),
                                )
                            if s == HS2 - 1:
                                ys = yp.tile([P, NT], f32, tag="y",
                                             name=f"y_{half}_{t}_{ob}")
                                nc.vector.tensor_copy(ys[:], pss[ob][:])
                                nc.sync.dma_start(
                                    yout[ob * P:(ob + 1) * P,
                                         t * NT:(t + 1) * NT],
                                    ys[:],
                                )
    nc.finalize()
    return nc


def _routing(hidden, router_w, router_b):
    """Top-2 routing, bit-matching the jax reference on CPU."""
    import jax
    import jax.numpy as jnp

    cpu = jax.local_devices(backend="cpu")[0]
    with jax.default_device(cpu):
        logits = jnp.einsum("bsh,he->bse", jnp.asarray(hidden),
                            jnp.asarray(router_w)) + jnp.asarray(router_b)
        probs = jax.nn.softmax(logits, axis=-1)
        tkp, tki = jax.lax.top_k(probs, TOP_K)
        tkp = tkp / jnp.sum(tkp, axis=-1, keepdims=True)
        tkp_np = np.asarray(tkp).reshape(T, TOP_K)
        tki_np = np.asarray(tki).reshape(T, TOP_K)
    return tkp_np, tki_np


def kernel(hidden_states, w1, b1, w2, b2, router_w, router_b):
    from concourse import bass_utils

    hidden_states = np.ascontiguousarray(hidden_states, dtype=np.float32)
    w1 = np.ascontiguousarray(w1, dtype=np.float32)
    b1 = np.ascontiguousarray(b1, dtype=np.float32)
    w2 = np.ascontiguousarray(w2, dtype=np.float32)
    b2 = np.ascontiguousarray(b2, dtype=np.float32)

    tkp, tki = _routing(hidden_states, router_w, router_b)
    x = hidden_states.reshape(T, H)

    idx_e, prob_e = [], []
    for e in range(NUM_EXPERTS):
        hit = tki == e                       # [T, 2] bool
        idx = np.nonzero(hit.any(axis=1))[0]
        pe = np.where(hit[idx, 0], tkp[idx, 0], tkp[idx, 1]).astype(np.float32)
        idx_e.append(idx)
        prob_e.append(pe)

    maxn = max(len(ix) for ix in idx_e)
    C = C_DEFAULT if maxn <= C_DEFAULT else ((maxn + NT - 1) // NT) * NT
    if C not in _built:
        _built[C] = _build(C)
    nc = _built[C]

    in_maps = []
    for e in range(NUM_EXPERTS):
        ix = idx_e[e]
        xt = np.zeros((H, C), dtype=np.float32)
        xt[:, :len(ix)] = x[ix].T
        in_maps.append({
            "xt": xt,
            "w1": w1[e],
            "b1": b1[e],
            "w2": w2[e],
        })

    res = bass_utils.run_bass_kernel_spmd(
        nc, in_maps, core_ids=list(range(NUM_EXPERTS))
    ).results

    out = np.zeros((T, H), dtype=np.float32)
    for e in range(NUM_EXPERTS):
        ix = idx_e[e]
        y = (res[e]["ya"][:, :len(ix)] + res[e]["yb"][:, :len(ix)]).T
        out[ix] += (y + b2[e]) * prob_e[e][:, None]
    return out.reshape(B, S, H)


# revision 9
# speedup vs baseline: 22.3686x; 8.6250x over previous
"""MoE (8 experts, top-2) Trainium2 kernel.

Strategy (per spec sharding_hint): expert parallelism. The host computes the
(cheap) router — logits, softmax, top-2, renormalized combine weights — and
dispatches each token to the cores owning its two experts ("all-to-all token
dispatch by top-k expert id" done at the sharding step, since kernel() holds
the full inputs host-side). Core e runs the expert-e FFN over its gathered
tokens, capacity-padded so all 8 cores run one SPMD program:

    YT = W2[e]^T @ gelu(W1[e]^T @ XT + b1[e])        (feature-major layouts)

Matmuls run in float32r (fp32 storage, full PE rate for moving dim >= 256).
The host then scatter-adds  (Y + b2[e]) * combine  back into the output.
"""

import os
import sys

import numpy as np

for _p in ("/opt/trn_rl_repo", "/root/.axon_site/_ro/trn_rl_repo"):
    if os.path.isdir(_p) and _p not in sys.path:
        sys.path.insert(0, _p)

NUM_EXPERTS = 8
TOP_K = 2
B, S, H, I = 4, 4096, 1024, 4096
T = B * S
P = 128
NT = 512           # max token tile = moving free dim (fp32 max 512)
C_DEFAULT = 4352   # capacity per expert (seed-0 max count 4302), mult of 256


def _token_tiles(C):
    """Split C into tiles of 512 plus at most one trailing 256."""
    assert C % 256 == 0
    tiles, off = [], 0
    while C - off >= 512:
        tiles.append((off, 512))
        off += 512
    if C - off:
        tiles.append((off, 256))
        off = C
    return tiles
KH = H // P        # 8 contraction chunks for stage 1
KI = I // P        # 32 contraction chunks for stage 2

_built = {}        # C -> (nc, input names)


def _build(C):
    import concourse.bacc as bacc
    import concourse.mybir as mybir
    import concourse.tile as tile
    from concourse._compat import get_trn_type

    f32 = mybir.dt.float32
    f32r = mybir.dt.float32r
    GELU = mybir.ActivationFunctionType.Gelu

    nc = bacc.Bacc(
        get_trn_type() or "TRN2",
        target_bir_lowering=False,
        debug=False,
        enable_asserts=False,
    )
    xt = nc.dram_tensor("xt", [H, C], f32r, kind="ExternalInput").ap()
    w1 = nc.dram_tensor("w1", [H, I], f32r, kind="ExternalInput").ap()
    b1 = nc.dram_tensor("b1", [I], f32, kind="ExternalInput").ap()
    w2 = nc.dram_tensor("w2", [I, H], f32r, kind="ExternalInput").ap()
    ya = nc.dram_tensor("ya", [H, C], f32, kind="ExternalOutput").ap()
    yb = nc.dram_tensor("yb", [H, C], f32, kind="ExternalOutput").ap()

    tiles = _token_tiles(C)
    IH = I // 2         # 2048: i-range per half-phase
    KIH = KI // 2       # 16 stage-2 contraction chunks per half
    XS = 2              # x sub-tiles per token tile (k-chunks split 2x4)
    HS2 = 4             # h2 sub-tiles per token tile (k-chunks split 4x4)

    with tile.TileContext(nc) as tc:
        with (
            tc.tile_pool(name="dram", bufs=1, space="DRAM") as drampool,
            tc.tile_pool(name="bias", bufs=1) as bpool,
            tc.tile_pool(name="wp", bufs=2) as wp,
            tc.tile_pool(name="xp", bufs=3) as xp,
            tc.tile_pool(name="hsp", bufs=3) as hsp,
            tc.tile_pool(name="h2p", bufs=3) as h2p,
            tc.tile_pool(name="yp", bufs=2) as yp,
            tc.tile_pool(name="psp", bufs=8, space="PSUM") as psp,
        ):
            hta = drampool.tile([KIH, P, C], f32r, tag="hta")
            htb = drampool.tile([KIH, P, C], f32r, tag="htb")
            b1sb = bpool.tile([P, KI], f32)
            nc.sync.dma_start(b1sb[:], b1.rearrange("(ib p) -> p ib", p=P))

            w1r = w1.rearrange("(ko p) i -> p ko i", p=P)
            w2r = w2.rearrange("(ko p) o -> p ko o", p=P)

            for half in range(2):
                ht = hta if half == 0 else htb
                yout = ya if half == 0 else yb

                # -- Phase 1x: HT[i,t] = gelu(W1[:,i]^T XT + b1[i]), i in half
                w1sb = wp.tile([P, KH, IH], f32r, tag="w")

                def _load_w1(lo, hi, half=half, w1sb=w1sb):
                    nc.sync.dma_start(
                        w1sb[:, :, lo:hi],
                        w1r[:, :, half * IH + lo: half * IH + hi],
                    )

                def _load_x(t, s, toff, tsz):
                    xst = xp.tile([P, KH // XS, tsz], f32r, tag="x",
                                  name=f"x_{half}_{t}_{s}")
                    nc.sync.dma_start(
                        xst[:],
                        xt[(s * KH // XS) * P:((s + 1) * KH // XS) * P,
                           toff:toff + tsz].rearrange(
                            "(ko p) n -> p ko n", p=P
                        ),
                    )
                    return xst

                # first w1 column chunk, then t=0 x tiles, then the rest —
                # so PE starts after ~2MB of DMA, not ~20MB
                _load_w1(0, 128)
                xs0 = [_load_x(0, s, tiles[0][0], tiles[0][1])
                       for s in range(XS)]
                for lo, hi in ((128, 256), (256, 384), (384, 512),
                               (512, 1024), (1024, 1536), (1536, 2048)):
                    _load_w1(lo, hi)
                for t, (toff, tsz) in enumerate(tiles):
                    xs = xs0 if t == 0 else [
                        _load_x(t, s, toff, tsz) for s in range(XS)]
                    for ibl in range(KIH):
                        ib = half * KIH + ibl
                        ps = psp.tile([P, tsz], f32, tag="ps",
                                      name=f"ps1_{half}_{t}_{ibl}")
                        for k in range(KH):
                            nc.tensor.matmul(
                                ps[:],
                                lhsT=w1sb[:, k, ibl * P:(ibl + 1) * P],
                                rhs=xs[k // (KH // XS)][:, k % (KH // XS)],
                                start=(k == 0),
                                stop=(k == KH - 1),
                            )
                        hs = hsp.tile([P, tsz], f32r, tag="hs",
                                      name=f"hs_{half}_{t}_{ibl}")
                        nc.scalar.activation(
                            hs[:], ps[:], GELU, bias=b1sb[:, ib:ib + 1]
                        )
                        nc.sync.dma_start(ht[ibl, :, toff:toff + tsz], hs[:])

                # -- Phase 2x: Y_half[o,t] = sum_{i in half} W2[i,o] HT[i,t]
                w2sb = wp.tile([P, KIH, H], f32r, tag="w")
                for c in range(4):
                    cw = KIH // 4
                    nc.sync.dma_start(
                        w2sb[:, c * cw:(c + 1) * cw],
                        w2r[:, half * KIH + c * cw: half * KIH + (c + 1) * cw],
                    )
                HC = KIH // HS2   # 4 chunks per sub-tile
                for t, (toff, tsz) in enumerate(tiles):
                    pss = []
                    for _ob in range(H // P):
                        pst = psp.tile([P, tsz], f32, tag="ps",
                                       name=f"ps_{half}_{t}_{_ob}")
                        pss.append(pst)
                    for s in range(HS2):
                        h2t = h2p.tile([P, HC, tsz], f32r, tag="h2",
                                       name=f"h2_{half}_{t}_{s}")
                        nc.sync.dma_start(
                            h2t[:],
                            ht[s * HC:(s + 1) * HC, :,
                               toff:toff + tsz].rearrange("ko p n -> p ko n"),
                        )
                        for ob in range(H // P):
                            for kk in range(HC):
                                nc.tensor.matmul(
                                    pss[ob][:],
                                    lhsT=w2sb[:, s * HC + kk,
                                              ob * P:(ob + 1) * P],
                                    rhs=h2t[:, kk],
                                    start=(s == 0 and kk == 0),
                                    stop=(s == HS2 - 1 and kk == HC - 1),
                                )
                            if s == HS2 - 1:
                                ys = yp.tile([P, tsz], f32, tag="y",
                                             name=f"y_{half}_{t}_{ob}")
                                nc.vector.tensor_copy(ys[:], pss[ob][:])
                                nc.sync.dma_start(
                                    yout[ob * P:(ob + 1) * P,
                                         toff:toff + tsz],
                                    ys[:],
                                )
    nc.finalize()
    return nc


def _routing(hidden, router_w, router_b):
    """Top-2 routing, bit-matching the jax reference on CPU."""
    import jax
    import jax.numpy as jnp

    cpu = jax.local_devices(backend="cpu")[0]
    with jax.default_device(cpu):
        logits = jnp.einsum("bsh,he->bse", jnp.asarray(hidden),
                            jnp.asarray(router_w)) + jnp.asarray(router_b)
        probs = jax.nn.softmax(logits, axis=-1)
        tkp, tki = jax.lax.top_k(probs, TOP_K)
        tkp = tkp / jnp.sum(tkp, axis=-1, keepdims=True)
        tkp_np = np.asarray(tkp).reshape(T, TOP_K)
        tki_np = np.asarray(tki).reshape(T, TOP_K)
    return tkp_np, tki_np


def kernel(hidden_states, w1, b1, w2, b2, router_w, router_b):
    from concourse import bass_utils

    hidden_states = np.ascontiguousarray(hidden_states, dtype=np.float32)
    w1 = np.ascontiguousarray(w1, dtype=np.float32)
    b1 = np.ascontiguousarray(b1, dtype=np.float32)
    w2 = np.ascontiguousarray(w2, dtype=np.float32)
    b2 = np.ascontiguousarray(b2, dtype=np.float32)

    tkp, tki = _routing(hidden_states, router_w, router_b)
    x = hidden_states.reshape(T, H)

    idx_e, prob_e = [], []
    for e in range(NUM_EXPERTS):
        hit = tki == e                       # [T, 2] bool
        idx = np.nonzero(hit.any(axis=1))[0]
        pe = np.where(hit[idx, 0], tkp[idx, 0], tkp[idx, 1]).astype(np.float32)
        idx_e.append(idx)
        prob_e.append(pe)

    maxn = max(len(ix) for ix in idx_e)
    C = C_DEFAULT if maxn <= C_DEFAULT else ((maxn + 255) // 256) * 256
    if C not in _built:
        _built[C] = _build(C)
    nc = _built[C]

    in_maps = []
    for e in range(NUM_EXPERTS):
        ix = idx_e[e]
        xt = np.zeros((H, C), dtype=np.float32)
        xt[:, :len(ix)] = x[ix].T
        in_maps.append({
            "xt": xt,
            "w1": w1[e],
            "b1": b1[e],
            "w2": w2[e],
        })

    res = bass_utils.run_bass_kernel_spmd(
        nc, in_maps, core_ids=list(range(NUM_EXPERTS))
    ).results

    out = np.zeros((T, H), dtype=np.float32)
    for e in range(NUM_EXPERTS):
        ix = idx_e[e]
        y = (res[e]["ya"][:, :len(ix)] + res[e]["yb"][:, :len(ix)]).T
        out[ix] += (y + b2[e]) * prob_e[e][:, None]
    return out.reshape(B, S, H)
